# revision 5
# baseline (speedup 1.0000x reference)
"""Trainium2 kernel for BinaryXnorExceptOutliersLinear.

Computes  out = x @ w_sim.T + bias  where
  w_sim = where(outlier_mask, weight, sign(weight) * binary_scale)

Distribution: column-parallel over 8 NeuronCores — weight / outlier_mask /
bias sharded along out_features (11008 -> 8 x 1376), x replicated, each core
produces its [8192, 1376] output slice, concatenated on host.

Layout strategy: all operands are staged on host into k-major, PE-ready
layouts so the tensor engine does NOTHING but the 6144 accumulating matmuls
per core (the bf16 roofline). binary_scale is folded into x on host
(x' = x*scale) and the DMA'd weight is pre-divided (w' = w/scale), so the
device-side weight prep is exactly two elementwise passes:
  wT = sign(w')          (ACT engine, inliers -> +-1)
  wT[mask] = w'[mask]    (DVE copy_predicated, outlier restore)
and then  out = x' @ wT + bias:  inliers contribute x*scale*sign(w),
outliers contribute x*w — identical to the reference up to bf16 rounding.

Per-core schedule:
  - weight prep runs chunk-major (3 out-feature chunks of 512/512/352) so
    the first matmul chunk is ready ~20us in; ACT computes signs, DVE the
    predicated outlier restore, all into a resident [128, 32, 1376] bf16 wT.
  - x streams in 16 blocks of 512 tokens ([128, 32, 512] bf16, 1KB runs)
    on the sync HWDGE ring, double-buffered.
  - warm phase: tiles 0-1 run chunk-by-chunk (only chunk-0 weights needed
    to start); steady state: tile-major, 96 matmuls per 128-token tile,
    kt-outer so the stationary x tile is reused across the 3 chunks.
  - DVE adds bias on the PSUM->SBUF drain; scalar-ring DMA writes out.
"""

import sys

for _p in ("/opt/trn_rl_repo",):
    if _p not in sys.path:
        sys.path.insert(0, _p)

import ml_dtypes
import numpy as np

import concourse.bass as bass
import concourse.mybir as mybir
from concourse.tile import TileContext
from concourse.bass_utils import run_bass_kernel_spmd

B, S, DIN, DOUT = 4, 2048, 4096, 11008
M = B * S              # 8192 tokens
NCORES = 8
DSH = DOUT // NCORES   # 1376 out-features per core
K = DIN
KT = K // 128          # 32 k-tiles
TB = 512               # tokens per x DMA block (4 token tiles)
CHUNKS = [(0, 512), (512, 512), (1024, 352)]   # out-feature chunks per core
QKT = 8                # k-tiles per weight-prep quarter

F32 = mybir.dt.float32
BF16 = mybir.dt.bfloat16
U8 = mybir.dt.uint8

MAX_WAITS = 1  # stock walrus: one sem-wait command per instruction


def _split_excess_waits(nc, max_waits: int = MAX_WAITS) -> int:
    """Stock AWS walrus rejects instructions with more than one sem-wait
    ("Too many sync wait commands"). Tile's kernel-tail drain waits on the
    final value of every proc's semaphore. Peel excess waits onto bare
    EventSemaphore stubs placed right before the instruction on the same
    engine (engines run their stream in order, so ordering is preserved)."""
    n_split = 0
    for f in nc.m.functions:
        for blk in f.blocks:
            il = blk.instructions
            out = []
            changed = False
            for inst in il:
                si = inst.sync_info
                waits = list(si.on_wait) if (si and si.on_wait) else []
                if len(waits) > max_waits:
                    changed = True
                    extra, keep = waits[:-max_waits], waits[-max_waits:]
                    for ci, start in enumerate(range(0, len(extra), max_waits)):
                        chunk = extra[start:start + max_waits]
                        stub = mybir.InstEventSemaphore(
                            name=f"{inst.name}_wsplit{ci}", ins=[], outs=[])
                        stub.engine = inst.engine
                        stub.sync_info = mybir.SyncInfo(
                            on_wait=list(chunk), on_update=[])
                        out.append(stub)
                        n_split += 1
                    si.on_wait = keep
                    inst.sync_info = si
                out.append(inst)
            if changed:
                il.clear()
                il.extend(out)
    return n_split


def build_nc(m_tokens: int = M):
    """Build the per-core Bass program (SPMD: same program on all cores)."""
    tok_tiles = m_tokens // 128
    n_blocks = (m_tokens + TB - 1) // TB
    tiles_per_blk = TB // 128
    nc = bass.Bass()
    # k-major host-staged layouts: [p, kt, ...] with p the SBUF partition
    xt_h = nc.declare_dram_parameter("xt", [128, KT, m_tokens], BF16,
                                     isOutput=False)
    wt_h = nc.declare_dram_parameter("wt", [128, KT, DSH], BF16,
                                     isOutput=False)
    mk_h = nc.declare_dram_parameter("mk", [128, KT, DSH], U8, isOutput=False)
    b_h = nc.declare_dram_parameter("bias", [DSH], F32, isOutput=False)
    out_h = nc.declare_dram_parameter("out", [m_tokens, DSH], F32,
                                      isOutput=True)

    with TileContext(nc) as tc:
        with tc.tile_pool(name="const", bufs=1) as const_pool, \
             tc.tile_pool(name="xp", bufs=2) as xp, \
             tc.tile_pool(name="wprep", bufs=2) as wp, \
             tc.tile_pool(name="osbp", bufs=2) as op, \
             tc.tile_pool(name="mpsum", bufs=2, space="PSUM") as psum_pool:

            # Resident binarized weight, k-major: [k-in-tile, kt, dout]
            wT = const_pool.tile([128, KT * DSH], BF16)
            wT_r = wT.rearrange("p (kt d) -> p kt d", kt=KT)
            bias_rep = const_pool.tile([128, DSH], F32)
            nc.gpsimd.dma_start(
                out=bias_rep,
                in_=b_h[:].rearrange("(a d) -> a d",
                                     a=1).to_broadcast((128, DSH)))

            # ---- x block DMAs (sync HWDGE ring, shared with out DMAs;
            #      emit first two upfront, the rest one block ahead) ----
            xbufs = {}

            def emit_xdma(b):
                xb = xp.tile([128, KT * TB], BF16, tag="xb", name="xb")
                xb_r = xb.rearrange("p (kt t) -> p kt t", kt=KT)
                nc.sync.dma_start(xb_r, xt_h[:, :, b * TB:(b + 1) * TB])
                xbufs[b] = xb_r

            # ---- weight prep, chunk-major, QKT k-tiles per step ----
            # wf holds w' = w/scale (bf16, host-staged); ACT writes
            # sign(w') straight into the resident wT slice, DVE restores
            # the outlier entries from wf under the mask. Ring split so
            # chunk 0 lands fast: weights on scalar ring (chunk 0's later
            # quarters on sync), masks on gpsimd; ACT/DVE run per
            # half-chunk so the first columns are ready early.
            def emit_prep(ci):
                coff, csz = CHUNKS[ci]
                for q in range(KT // QKT):
                    ks = slice(q * QKT, (q + 1) * QKT)
                    wf = wp.tile([128, QKT * 512], BF16, tag="wf", name="wf")
                    mk = wp.tile([128, QKT * 512], U8, tag="mk", name="mk")
                    wf_r = wf.rearrange("p (kt d) -> p kt d", kt=QKT)
                    mk_r = mk.rearrange("p (kt d) -> p kt d", kt=QKT)
                    weng = nc.sync if (ci == 0 and q % 2 == 1) else nc.scalar
                    weng.dma_start(wf_r[:, :, :csz],
                                   wt_h[:, ks, coff:coff + csz])
                    nc.gpsimd.dma_start(mk_r[:, :, :csz],
                                        mk_h[:, ks, coff:coff + csz])
                    nc.scalar.sign(wT_r[:, ks, coff:coff + csz],
                                   wf_r[:, :, :csz])
                    nc.vector.copy_predicated(wT_r[:, ks, coff:coff + csz],
                                              mk_r[:, :, :csz],
                                              wf_r[:, :, :csz])

            emit_prep(0)
            emit_xdma(0)
            if n_blocks > 1:
                emit_xdma(1)
            emit_prep(1)
            emit_prep(2)

            # ---- matmul emission helpers ----
            def lhsT(t, kt):
                b, j = divmod(t, tiles_per_blk)
                return xbufs[b][:, kt, j * 128:(j + 1) * 128]

            def drain_out(t, ci, ps, coff, csz):
                """PSUM -> (+bias) -> SBUF chunk tile -> DRAM columns."""
                osb = op.tile([128, 512], F32, tag=f"osb{ci % 3}",
                              name="osb")
                nc.vector.tensor_add(osb[:, :csz], ps[:, :csz],
                                     bias_rep[:, coff:coff + csz])
                nc.sync.dma_start(
                    out_h[t * 128:(t + 1) * 128, coff:coff + csz],
                    osb[:, :csz])

            def mm_group(t, ci, coff, csz, tag):
                ps = psum_pool.tile([128, 512], F32, tag=tag, name=tag)
                for kt in range(KT):
                    nc.tensor.matmul(ps[:, :csz], lhsT(t, kt),
                                     wT_r[:, kt, coff:coff + csz],
                                     start=(kt == 0), stop=(kt == KT - 1))
                drain_out(t, ci, ps, coff, csz)

            # ---- warm phase: first tiles chunk-by-chunk, so the PE
            #      starts as soon as chunk 0's weights are prepped ----
            WARM = min(3, tok_tiles)
            for ci, (coff, csz) in enumerate(CHUNKS):
                for t in range(WARM):
                    mm_group(t, ci, coff, csz, f"pso{ci}")

            # ---- steady state: tile-major, kt-outer / chunk-inner ----
            for t in range(WARM, tok_tiles):
                b, j = divmod(t, tiles_per_blk)
                if j == 0 and b + 1 < n_blocks and b + 1 not in xbufs:
                    emit_xdma(b + 1)
                psos = [psum_pool.tile([128, 512], F32, tag=f"pso{ci}",
                                       name=f"pso{ci}")
                        for ci in range(len(CHUNKS))]
                for kt in range(KT):
                    xk = lhsT(t, kt)
                    for ci, (coff, csz) in enumerate(CHUNKS):
                        nc.tensor.matmul(psos[ci][:, :csz], xk,
                                         wT_r[:, kt, coff:coff + csz],
                                         start=(kt == 0), stop=(kt == KT - 1))
                for ci, (coff, csz) in enumerate(CHUNKS):
                    drain_out(t, ci, psos[ci], coff, csz)

    _split_excess_waits(nc)
    return nc


_NC_CACHE = {}


def _get_nc(m_tokens: int = M):
    if m_tokens not in _NC_CACHE:
        _NC_CACHE[m_tokens] = build_nc(m_tokens)
    return _NC_CACHE[m_tokens]


def _kmajor(a2d):
    """[R, C] (R = DIN) -> contiguous [128, R//128, C] k-major layout."""
    r, c = a2d.shape
    return np.ascontiguousarray(
        a2d.reshape(r // 128, 128, c).transpose(1, 0, 2))


def _make_in_maps(x, weight, bias, outlier_mask, binary_scale):
    m_tokens = x.shape[0] * x.shape[1] if x.ndim == 3 else x.shape[0]
    scale = np.float32(np.asarray(binary_scale).reshape(-1)[0])
    bf16 = ml_dtypes.bfloat16
    # fold binary_scale into x; pre-divide the weight so the device's
    # sign + outlier-restore produce w' with inliers +-1, outliers w/scale
    xs = (np.asarray(x, dtype=np.float32).reshape(m_tokens, K)
          * scale).astype(bf16)
    xt = _kmajor(np.ascontiguousarray(xs.T))          # [128, KT, M]
    w = np.asarray(weight, dtype=np.float32)
    b = np.asarray(bias, dtype=np.float32)
    mku = np.ascontiguousarray(outlier_mask).view(np.uint8)
    in_maps = []
    for i in range(NCORES):
        sl = slice(i * DSH, (i + 1) * DSH)
        wq = (w[sl] / scale).astype(bf16)             # [DSH, K]
        in_maps.append({
            "xt": xt,
            "wt": _kmajor(np.ascontiguousarray(wq.T)),        # [128, KT, DSH]
            "mk": _kmajor(np.ascontiguousarray(mku[sl].T)),   # [128, KT, DSH]
            "bias": np.ascontiguousarray(b[sl]),
        })
    return in_maps, m_tokens


def run_sharded(x, weight, bias, outlier_mask, binary_scale, trace=False):
    """Run on 8 cores; returns (full_output [M, DOUT] f32, BassKernelResults)."""
    in_maps, m_tokens = _make_in_maps(x, weight, bias, outlier_mask,
                                      binary_scale)
    nc = _get_nc(m_tokens)
    res = run_bass_kernel_spmd(nc, in_maps, core_ids=list(range(NCORES)),
                               trace=trace)
    full = np.concatenate([res.results[i]["out"] for i in range(NCORES)],
                          axis=1)
    return full, res


def kernel(x, weight, bias, outlier_mask, binary_scale):
    full, _ = run_sharded(x, weight, bias, outlier_mask, binary_scale)
    return full.reshape(x.shape[0], x.shape[1], DOUT) if x.ndim == 3 else full


# revision 9
# speedup vs baseline: 1.1933x; 1.1933x over previous
"""Trainium2 kernel for BinaryXnorExceptOutliersLinear.

Computes  out = x @ w_sim.T + bias  where
  w_sim = where(outlier_mask, weight, sign(weight) * binary_scale)

Distribution: column-parallel over 8 NeuronCores — weight / outlier_mask /
bias sharded along out_features (11008 -> 8 x 1376), x replicated, each core
produces its [8192, 1376] output slice, concatenated on host.

Layout strategy: all operands are staged on host into k-major, PE-ready
layouts so the tensor engine does NOTHING but the 6144 accumulating matmuls
per core (the bf16 roofline). binary_scale is folded into x on host
(x' = x*scale) and the DMA'd weight is pre-divided (w' = w/scale), so the
device-side weight prep is exactly two elementwise passes:
  wT = sign(w')          (ACT engine, inliers -> +-1)
  wT[mask] = w'[mask]    (DVE copy_predicated, outlier restore)
and then  out = x' @ wT + bias:  inliers contribute x*scale*sign(w),
outliers contribute x*w — identical to the reference up to bf16 rounding.

Per-core schedule:
  - weight prep runs chunk-major (3 out-feature chunks of 512/512/352) so
    the first matmul chunk is ready ~20us in; ACT computes signs, DVE the
    predicated outlier restore, all into a resident [128, 32, 1376] bf16 wT.
  - x streams in 16 blocks of 512 tokens ([128, 32, 512] bf16, 1KB runs)
    on the sync HWDGE ring, double-buffered.
  - warm phase: tiles 0-1 run chunk-by-chunk (only chunk-0 weights needed
    to start); steady state: tile-major, 96 matmuls per 128-token tile,
    kt-outer so the stationary x tile is reused across the 3 chunks.
  - DVE adds bias on the PSUM->SBUF drain; scalar-ring DMA writes out.
"""

import sys

for _p in ("/opt/trn_rl_repo",):
    if _p not in sys.path:
        sys.path.insert(0, _p)

import ml_dtypes
import numpy as np

import concourse.bass as bass
import concourse.mybir as mybir
from concourse.tile import TileContext
from concourse.bass_utils import run_bass_kernel_spmd

B, S, DIN, DOUT = 4, 2048, 4096, 11008
M = B * S              # 8192 tokens
NCORES = 8
DSH = DOUT // NCORES   # 1376 out-features per core
K = DIN
KT = K // 128          # 32 k-tiles
TB = 512               # tokens per x DMA block (4 token tiles)
CHUNKS = [(0, 512), (512, 512), (1024, 352)]   # out-feature chunks per core
QKT = 8                # k-tiles per weight-prep quarter

F32 = mybir.dt.float32
BF16 = mybir.dt.bfloat16
U8 = mybir.dt.uint8

MAX_WAITS = 1  # stock walrus: one sem-wait command per instruction


def _split_excess_waits(nc, max_waits: int = MAX_WAITS) -> int:
    """Stock AWS walrus rejects instructions with more than one sem-wait
    ("Too many sync wait commands"). Tile's kernel-tail drain waits on the
    final value of every proc's semaphore. Peel excess waits onto bare
    EventSemaphore stubs placed right before the instruction on the same
    engine (engines run their stream in order, so ordering is preserved)."""
    n_split = 0
    for f in nc.m.functions:
        for blk in f.blocks:
            il = blk.instructions
            out = []
            changed = False
            for inst in il:
                si = inst.sync_info
                waits = list(si.on_wait) if (si and si.on_wait) else []
                if len(waits) > max_waits:
                    changed = True
                    extra, keep = waits[:-max_waits], waits[-max_waits:]
                    for ci, start in enumerate(range(0, len(extra), max_waits)):
                        chunk = extra[start:start + max_waits]
                        stub = mybir.InstEventSemaphore(
                            name=f"{inst.name}_wsplit{ci}", ins=[], outs=[])
                        stub.engine = inst.engine
                        stub.sync_info = mybir.SyncInfo(
                            on_wait=list(chunk), on_update=[])
                        out.append(stub)
                        n_split += 1
                    si.on_wait = keep
                    inst.sync_info = si
                out.append(inst)
            if changed:
                il.clear()
                il.extend(out)
    return n_split


def build_nc(m_tokens: int = M):
    """Build the per-core Bass program (SPMD: same program on all cores)."""
    tok_tiles = m_tokens // 128
    n_blocks = (m_tokens + TB - 1) // TB
    tiles_per_blk = TB // 128
    nc = bass.Bass()
    # k-major host-staged layouts: [p, kt, ...] with p the SBUF partition
    xt_h = nc.declare_dram_parameter("xt", [128, KT, m_tokens], BF16,
                                     isOutput=False)
    wt_h = nc.declare_dram_parameter("wt", [128, KT, DSH], BF16,
                                     isOutput=False)
    mk_h = nc.declare_dram_parameter("mk", [128, KT, DSH], U8, isOutput=False)
    b_h = nc.declare_dram_parameter("bias", [DSH], F32, isOutput=False)
    out_h = nc.declare_dram_parameter("out", [m_tokens, DSH], F32,
                                      isOutput=True)

    with TileContext(nc) as tc:
        with tc.tile_pool(name="const", bufs=1) as const_pool, \
             tc.tile_pool(name="xp", bufs=2) as xp, \
             tc.tile_pool(name="wprep", bufs=2) as wp, \
             tc.tile_pool(name="osbp", bufs=2) as op, \
             tc.tile_pool(name="mpsum", bufs=2, space="PSUM") as psum_pool:

            # Resident binarized weight, k-major: [k-in-tile, kt, dout]
            wT = const_pool.tile([128, KT * DSH], BF16)
            wT_r = wT.rearrange("p (kt d) -> p kt d", kt=KT)
            bias_rep = const_pool.tile([128, DSH], F32)
            nc.gpsimd.dma_start(
                out=bias_rep,
                in_=b_h[:].rearrange("(a d) -> a d",
                                     a=1).to_broadcast((128, DSH)))

            # ---- x block DMAs (sync HWDGE ring, shared with out DMAs;
            #      emit first two upfront, the rest one block ahead) ----
            xbufs = {}

            def emit_xdma(b, nsplit=1):
                xb = xp.tile([128, KT * TB], BF16, tag="xb", name="xb")
                xb_r = xb.rearrange("p (kt t) -> p kt t", kt=KT)
                ts = slice(b * TB, (b + 1) * TB)
                for s in range(nsplit):
                    ks = slice(s * (KT // nsplit), (s + 1) * (KT // nsplit))
                    nc.sync.dma_start(xb_r[:, ks, :], xt_h[:, ks, ts])
                xbufs[b] = xb_r

            # ---- weight prep, chunk-major, QKT k-tiles per step ----
            # wf holds w' = w/scale (bf16, host-staged); ACT writes
            # sign(w') straight into the resident wT slice, DVE restores
            # the outlier entries from wf under the mask. Ring split so
            # chunk 0 lands fast: weights on scalar ring (chunk 0's later
            # quarters on sync), masks on gpsimd; ACT/DVE run per
            # half-chunk so the first columns are ready early.
            def emit_prep(ci):
                coff, csz = CHUNKS[ci]
                for q in range(KT // QKT):
                    ks = slice(q * QKT, (q + 1) * QKT)
                    wf = wp.tile([128, QKT * 512], BF16, tag="wf", name="wf")
                    mk = wp.tile([128, QKT * 512], U8, tag="mk", name="mk")
                    wf_r = wf.rearrange("p (kt d) -> p kt d", kt=QKT)
                    mk_r = mk.rearrange("p (kt d) -> p kt d", kt=QKT)
                    nc.scalar.dma_start(wf_r[:, :, :csz],
                                        wt_h[:, ks, coff:coff + csz])
                    nc.gpsimd.dma_start(mk_r[:, :, :csz],
                                        mk_h[:, ks, coff:coff + csz])
                    nc.scalar.sign(wT_r[:, ks, coff:coff + csz],
                                   wf_r[:, :, :csz])
                    nc.vector.copy_predicated(wT_r[:, ks, coff:coff + csz],
                                              mk_r[:, :, :csz],
                                              wf_r[:, :, :csz])

            emit_prep(0)
            emit_xdma(0, nsplit=4)
            emit_prep(1)
            emit_prep(2)

            # ---- matmul emission helpers ----
            def lhsT(t, kt):
                b, j = divmod(t, tiles_per_blk)
                return xbufs[b][:, kt, j * 128:(j + 1) * 128]

            def drain_out(t, ci, ps, coff, csz):
                """PSUM -> (+bias) -> SBUF chunk tile -> DRAM columns."""
                osb = op.tile([128, 512], F32, tag=f"osb{ci % 3}",
                              name="osb")
                nc.vector.tensor_add(osb[:, :csz], ps[:, :csz],
                                     bias_rep[:, coff:coff + csz])
                nc.sync.dma_start(
                    out_h[t * 128:(t + 1) * 128, coff:coff + csz],
                    osb[:, :csz])

            def mm_group(t, ci, coff, csz, tag):
                ps = psum_pool.tile([128, 512], F32, tag=tag, name=tag)
                for kt in range(KT):
                    nc.tensor.matmul(ps[:, :csz], lhsT(t, kt),
                                     wT_r[:, kt, coff:coff + csz],
                                     start=(kt == 0), stop=(kt == KT - 1))
                drain_out(t, ci, ps, coff, csz)

            # ---- warm phase: first tiles chunk-by-chunk, so the PE
            #      starts as soon as chunk 0's weights are prepped ----
            WARM = min(3, tok_tiles)
            for ci, (coff, csz) in enumerate(CHUNKS):
                for t in range(WARM):
                    mm_group(t, ci, coff, csz, f"pso{ci}")
                if ci == 0 and n_blocks > 1:
                    # x block 1 lands behind the first warm outputs, after
                    # the startup-critical chunk-0 + x0 bytes have drained
                    emit_xdma(1)

            # ---- steady state: tile-major, kt-outer / chunk-inner ----
            for t in range(WARM, tok_tiles):
                b, j = divmod(t, tiles_per_blk)
                if j == 0 and b + 1 < n_blocks and b + 1 not in xbufs:
                    emit_xdma(b + 1)
                psos = [psum_pool.tile([128, 512], F32, tag=f"pso{ci}",
                                       name=f"pso{ci}")
                        for ci in range(len(CHUNKS))]
                for kt in range(KT):
                    xk = lhsT(t, kt)
                    for ci, (coff, csz) in enumerate(CHUNKS):
                        nc.tensor.matmul(psos[ci][:, :csz], xk,
                                         wT_r[:, kt, coff:coff + csz],
                                         start=(kt == 0), stop=(kt == KT - 1))
                for ci, (coff, csz) in enumerate(CHUNKS):
                    drain_out(t, ci, psos[ci], coff, csz)

    _split_excess_waits(nc)
    return nc


_NC_CACHE = {}


def _get_nc(m_tokens: int = M):
    if m_tokens not in _NC_CACHE:
        _NC_CACHE[m_tokens] = build_nc(m_tokens)
    return _NC_CACHE[m_tokens]


def _kmajor(a2d):
    """[R, C] (R = DIN) -> contiguous [128, R//128, C] k-major layout."""
    r, c = a2d.shape
    return np.ascontiguousarray(
        a2d.reshape(r // 128, 128, c).transpose(1, 0, 2))


def _make_in_maps(x, weight, bias, outlier_mask, binary_scale):
    m_tokens = x.shape[0] * x.shape[1] if x.ndim == 3 else x.shape[0]
    scale = np.float32(np.asarray(binary_scale).reshape(-1)[0])
    bf16 = ml_dtypes.bfloat16
    # fold binary_scale into x; pre-divide the weight so the device's
    # sign + outlier-restore produce w' with inliers +-1, outliers w/scale
    xs = (np.asarray(x, dtype=np.float32).reshape(m_tokens, K)
          * scale).astype(bf16)
    xt = _kmajor(np.ascontiguousarray(xs.T))          # [128, KT, M]
    w = np.asarray(weight, dtype=np.float32)
    b = np.asarray(bias, dtype=np.float32)
    mku = np.ascontiguousarray(outlier_mask).view(np.uint8)
    in_maps = []
    for i in range(NCORES):
        sl = slice(i * DSH, (i + 1) * DSH)
        wq = (w[sl] / scale).astype(bf16)             # [DSH, K]
        in_maps.append({
            "xt": xt,
            "wt": _kmajor(np.ascontiguousarray(wq.T)),        # [128, KT, DSH]
            "mk": _kmajor(np.ascontiguousarray(mku[sl].T)),   # [128, KT, DSH]
            "bias": np.ascontiguousarray(b[sl]),
        })
    return in_maps, m_tokens


def run_sharded(x, weight, bias, outlier_mask, binary_scale, trace=False):
    """Run on 8 cores; returns (full_output [M, DOUT] f32, BassKernelResults)."""
    in_maps, m_tokens = _make_in_maps(x, weight, bias, outlier_mask,
                                      binary_scale)
    nc = _get_nc(m_tokens)
    res = run_bass_kernel_spmd(nc, in_maps, core_ids=list(range(NCORES)),
                               trace=trace)
    full = np.concatenate([res.results[i]["out"] for i in range(NCORES)],
                          axis=1)
    return full, res


def kernel(x, weight, bias, outlier_mask, binary_scale):
    full, _ = run_sharded(x, weight, bias, outlier_mask, binary_scale)
    return full.reshape(x.shape[0], x.shape[1], DOUT) if x.ndim == 3 else full


# revision 11
# speedup vs baseline: 1.2006x; 1.0061x over previous
"""Trainium2 kernel for BinaryXnorExceptOutliersLinear.

Computes  out = x @ w_sim.T + bias  where
  w_sim = where(outlier_mask, weight, sign(weight) * binary_scale)

Distribution: column-parallel over 8 NeuronCores — weight / outlier_mask /
bias sharded along out_features (11008 -> 8 x 1376), x replicated, each core
produces its [8192, 1376] output slice, concatenated on host.

Layout strategy: all operands are staged on host into k-major, PE-ready
layouts so the tensor engine does NOTHING but the 6144 accumulating matmuls
per core (the bf16 roofline). binary_scale is folded into x on host
(x' = x*scale) and the DMA'd weight is pre-divided (w' = w/scale), so the
device-side weight prep is exactly two elementwise passes:
  wT = sign(w')          (ACT engine, inliers -> +-1)
  wT[mask] = w'[mask]    (DVE copy_predicated, outlier restore)
and then  out = x' @ wT + bias:  inliers contribute x*scale*sign(w),
outliers contribute x*w — identical to the reference up to bf16 rounding.

Per-core schedule:
  - weight prep runs chunk-major (3 out-feature chunks of 512/512/352) so
    the first matmul chunk is ready ~20us in; ACT computes signs, DVE the
    predicated outlier restore, all into a resident [128, 32, 1376] bf16 wT.
  - x streams in 16 blocks of 512 tokens ([128, 32, 512] bf16, 1KB runs)
    on the sync HWDGE ring, double-buffered.
  - warm phase: tiles 0-1 run chunk-by-chunk (only chunk-0 weights needed
    to start); steady state: tile-major, 96 matmuls per 128-token tile,
    kt-outer so the stationary x tile is reused across the 3 chunks.
  - DVE adds bias on the PSUM->SBUF drain; scalar-ring DMA writes out.
"""

import sys

for _p in ("/opt/trn_rl_repo",):
    if _p not in sys.path:
        sys.path.insert(0, _p)

import ml_dtypes
import numpy as np

import concourse.bass as bass
import concourse.mybir as mybir
from concourse.tile import TileContext
from concourse.bass_utils import run_bass_kernel_spmd

B, S, DIN, DOUT = 4, 2048, 4096, 11008
M = B * S              # 8192 tokens
NCORES = 8
DSH = DOUT // NCORES   # 1376 out-features per core
K = DIN
KT = K // 128          # 32 k-tiles
TB = 512               # tokens per x DMA block (4 token tiles)
CHUNKS = [(0, 512), (512, 512), (1024, 352)]   # out-feature chunks per core
QKT = 8                # k-tiles per weight-prep quarter

F32 = mybir.dt.float32
BF16 = mybir.dt.bfloat16
U8 = mybir.dt.uint8

MAX_WAITS = 1  # stock walrus: one sem-wait command per instruction


def _split_excess_waits(nc, max_waits: int = MAX_WAITS) -> int:
    """Stock AWS walrus rejects instructions with more than one sem-wait
    ("Too many sync wait commands"). Tile's kernel-tail drain waits on the
    final value of every proc's semaphore. Peel excess waits onto bare
    EventSemaphore stubs placed right before the instruction on the same
    engine (engines run their stream in order, so ordering is preserved)."""
    n_split = 0
    for f in nc.m.functions:
        for blk in f.blocks:
            il = blk.instructions
            out = []
            changed = False
            for inst in il:
                si = inst.sync_info
                waits = list(si.on_wait) if (si and si.on_wait) else []
                if len(waits) > max_waits:
                    changed = True
                    extra, keep = waits[:-max_waits], waits[-max_waits:]
                    for ci, start in enumerate(range(0, len(extra), max_waits)):
                        chunk = extra[start:start + max_waits]
                        stub = mybir.InstEventSemaphore(
                            name=f"{inst.name}_wsplit{ci}", ins=[], outs=[])
                        stub.engine = inst.engine
                        stub.sync_info = mybir.SyncInfo(
                            on_wait=list(chunk), on_update=[])
                        out.append(stub)
                        n_split += 1
                    si.on_wait = keep
                    inst.sync_info = si
                out.append(inst)
            if changed:
                il.clear()
                il.extend(out)
    return n_split


def build_nc(m_tokens: int = M):
    """Build the per-core Bass program (SPMD: same program on all cores)."""
    tok_tiles = m_tokens // 128
    n_blocks = (m_tokens + TB - 1) // TB
    tiles_per_blk = TB // 128
    nc = bass.Bass()
    # k-major host-staged layouts: [p, kt, ...] with p the SBUF partition
    xt_h = nc.declare_dram_parameter("xt", [128, KT, m_tokens], BF16,
                                     isOutput=False)
    wt_h = nc.declare_dram_parameter("wt", [128, KT, DSH], BF16,
                                     isOutput=False)
    mk_h = nc.declare_dram_parameter("mk", [128, KT, DSH], U8, isOutput=False)
    b_h = nc.declare_dram_parameter("bias", [DSH], F32, isOutput=False)
    out_h = nc.declare_dram_parameter("out", [m_tokens, DSH], F32,
                                      isOutput=True)

    with TileContext(nc) as tc:
        with tc.tile_pool(name="const", bufs=1) as const_pool, \
             tc.tile_pool(name="xp", bufs=2) as xp, \
             tc.tile_pool(name="wprep", bufs=2) as wp, \
             tc.tile_pool(name="osbp", bufs=2) as op, \
             tc.tile_pool(name="mpsum", bufs=2, space="PSUM") as psum_pool:

            # Resident binarized weight, k-major: [k-in-tile, kt, dout]
            wT = const_pool.tile([128, KT * DSH], BF16)
            wT_r = wT.rearrange("p (kt d) -> p kt d", kt=KT)
            bias_rep = const_pool.tile([128, DSH], F32)
            nc.gpsimd.dma_start(
                out=bias_rep,
                in_=b_h[:].rearrange("(a d) -> a d",
                                     a=1).to_broadcast((128, DSH)))

            # ---- x block DMAs (sync HWDGE ring, shared with out DMAs;
            #      emit first two upfront, the rest one block ahead) ----
            xbufs = {}

            def emit_xdma(b, nsplit=1):
                xb = xp.tile([128, KT * TB], BF16, tag="xb", name="xb")
                xb_r = xb.rearrange("p (kt t) -> p kt t", kt=KT)
                ts = slice(b * TB, (b + 1) * TB)
                for s in range(nsplit):
                    ks = slice(s * (KT // nsplit), (s + 1) * (KT // nsplit))
                    nc.sync.dma_start(xb_r[:, ks, :], xt_h[:, ks, ts])
                xbufs[b] = xb_r

            # ---- weight prep, chunk-major, QKT k-tiles per step ----
            # wf holds w' = w/scale (bf16, host-staged); ACT writes
            # sign(w') straight into the resident wT slice, DVE restores
            # the outlier entries from wf under the mask. Ring split so
            # chunk 0 lands fast: weights on scalar ring (chunk 0's later
            # quarters on sync), masks on gpsimd; ACT/DVE run per
            # half-chunk so the first columns are ready early.
            def emit_prep(ci):
                coff, csz = CHUNKS[ci]
                for q in range(KT // QKT):
                    ks = slice(q * QKT, (q + 1) * QKT)
                    wf = wp.tile([128, QKT * 512], BF16, tag="wf", name="wf")
                    mk = wp.tile([128, QKT * 512], U8, tag="mk", name="mk")
                    wf_r = wf.rearrange("p (kt d) -> p kt d", kt=QKT)
                    mk_r = mk.rearrange("p (kt d) -> p kt d", kt=QKT)
                    nc.scalar.dma_start(wf_r[:, :, :csz],
                                        wt_h[:, ks, coff:coff + csz])
                    nc.gpsimd.dma_start(mk_r[:, :, :csz],
                                        mk_h[:, ks, coff:coff + csz])
                    nc.scalar.sign(wT_r[:, ks, coff:coff + csz],
                                   wf_r[:, :, :csz])
                    nc.vector.copy_predicated(wT_r[:, ks, coff:coff + csz],
                                              mk_r[:, :, :csz],
                                              wf_r[:, :, :csz])

            emit_prep(0)
            emit_xdma(0, nsplit=4)
            emit_prep(1)
            emit_prep(2)

            # ---- matmul emission helpers ----
            def lhsT(t, kt):
                b, j = divmod(t, tiles_per_blk)
                return xbufs[b][:, kt, j * 128:(j + 1) * 128]

            def drain_out(t, ci, ps, coff, csz):
                """PSUM -> (+bias) -> SBUF chunk tile -> DRAM columns."""
                osb = op.tile([128, 512], F32, tag=f"osb{ci % 3}",
                              name="osb")
                nc.vector.tensor_add(osb[:, :csz], ps[:, :csz],
                                     bias_rep[:, coff:coff + csz])
                nc.sync.dma_start(
                    out_h[t * 128:(t + 1) * 128, coff:coff + csz],
                    osb[:, :csz])

            def mm_group(t, ci, coff, csz, tag):
                ps = psum_pool.tile([128, 512], F32, tag=tag, name=tag)
                for kt in range(KT):
                    nc.tensor.matmul(ps[:, :csz], lhsT(t, kt),
                                     wT_r[:, kt, coff:coff + csz],
                                     start=(kt == 0), stop=(kt == KT - 1))
                drain_out(t, ci, ps, coff, csz)

            # ---- warm phase: first tiles chunk-by-chunk, so the PE
            #      starts as soon as chunk 0's weights are prepped ----
            WARM = min(6, tok_tiles)
            for ci, (coff, csz) in enumerate(CHUNKS):
                for t in range(WARM):
                    mm_group(t, ci, coff, csz, f"pso{ci}")
                    if ci == 0 and t == 0 and n_blocks > 1:
                        # x block 1 right after the first warm output: off
                        # the startup-critical window, but ahead of the
                        # warm outs of tiles 4-5 that depend on it (a
                        # later slot would deadlock the sync FIFO)
                        emit_xdma(1)

            # ---- steady state: tile-major, kt-outer / chunk-inner ----
            for t in range(WARM, tok_tiles):
                b, j = divmod(t, tiles_per_blk)
                if b not in xbufs:
                    emit_xdma(b)
                if j == 0 and b + 1 < n_blocks and b + 1 not in xbufs:
                    emit_xdma(b + 1)
                psos = [psum_pool.tile([128, 512], F32, tag=f"pso{ci}",
                                       name=f"pso{ci}")
                        for ci in range(len(CHUNKS))]
                for kt in range(KT):
                    xk = lhsT(t, kt)
                    for ci, (coff, csz) in enumerate(CHUNKS):
                        nc.tensor.matmul(psos[ci][:, :csz], xk,
                                         wT_r[:, kt, coff:coff + csz],
                                         start=(kt == 0), stop=(kt == KT - 1))
                for ci, (coff, csz) in enumerate(CHUNKS):
                    drain_out(t, ci, psos[ci], coff, csz)

    _split_excess_waits(nc)
    return nc


_NC_CACHE = {}


def _get_nc(m_tokens: int = M):
    if m_tokens not in _NC_CACHE:
        _NC_CACHE[m_tokens] = build_nc(m_tokens)
    return _NC_CACHE[m_tokens]


def _kmajor(a2d):
    """[R, C] (R = DIN) -> contiguous [128, R//128, C] k-major layout."""
    r, c = a2d.shape
    return np.ascontiguousarray(
        a2d.reshape(r // 128, 128, c).transpose(1, 0, 2))


def _make_in_maps(x, weight, bias, outlier_mask, binary_scale):
    m_tokens = x.shape[0] * x.shape[1] if x.ndim == 3 else x.shape[0]
    scale = np.float32(np.asarray(binary_scale).reshape(-1)[0])
    bf16 = ml_dtypes.bfloat16
    # fold binary_scale into x; pre-divide the weight so the device's
    # sign + outlier-restore produce w' with inliers +-1, outliers w/scale
    xs = (np.asarray(x, dtype=np.float32).reshape(m_tokens, K)
          * scale).astype(bf16)
    xt = _kmajor(np.ascontiguousarray(xs.T))          # [128, KT, M]
    w = np.asarray(weight, dtype=np.float32)
    b = np.asarray(bias, dtype=np.float32)
    mku = np.ascontiguousarray(outlier_mask).view(np.uint8)
    in_maps = []
    for i in range(NCORES):
        sl = slice(i * DSH, (i + 1) * DSH)
        wq = (w[sl] / scale).astype(bf16)             # [DSH, K]
        in_maps.append({
            "xt": xt,
            "wt": _kmajor(np.ascontiguousarray(wq.T)),        # [128, KT, DSH]
            "mk": _kmajor(np.ascontiguousarray(mku[sl].T)),   # [128, KT, DSH]
            "bias": np.ascontiguousarray(b[sl]),
        })
    return in_maps, m_tokens


def run_sharded(x, weight, bias, outlier_mask, binary_scale, trace=False):
    """Run on 8 cores; returns (full_output [M, DOUT] f32, BassKernelResults)."""
    in_maps, m_tokens = _make_in_maps(x, weight, bias, outlier_mask,
                                      binary_scale)
    nc = _get_nc(m_tokens)
    res = run_bass_kernel_spmd(nc, in_maps, core_ids=list(range(NCORES)),
                               trace=trace)
    full = np.concatenate([res.results[i]["out"] for i in range(NCORES)],
                          axis=1)
    return full, res


def kernel(x, weight, bias, outlier_mask, binary_scale):
    full, _ = run_sharded(x, weight, bias, outlier_mask, binary_scale)
    return full.reshape(x.shape[0], x.shape[1], DOUT) if x.ndim == 3 else full
